# revision 14
# baseline (speedup 1.0000x reference)
"""Trainium2 Bass kernel for nn_DifferentiableRobustMVOLayer.

Solves, per batch instance (512 total, 64 per core across 8 cores):
    min_w  -mu'w + kappa*||A w|| + (lambd/2)*||U w||^2   s.t. w in simplex
via 400 unrolled PGD iterations, matching the jax reference trajectory.

Per-core structure (64 instances, n=256):
  - 2 sequential chunks of 32 instances (fp32 G1/G2 for 32 instances fill
    ~16.8MB of the 24MB SBUF).
  - Precompute G1=U^T U, G2=A^T A with plain-fp32 PE matmuls (accurate,
    one-time), plus Frobenius norms for the fixed PGD step size.
  - 400-iteration dynamic loop. Each iteration:
      * 128 fp32r matvec matmuls: G streamed as the moving operand at
        1 cycle/row; the stationary is a per-instance "diagonal" block of
        w vectors so all 32 instances accumulate into [16,256] PSUM tiles
        (two 16-instance subgroups so the epilogue of one subgroup hides
        under the PE stream of the other).
      * Epilogue on DVE/ACT: robust norm term, gradient combine, and the
        simplex projection via warm-started Newton on
        f(tau)=sum(relu(v-tau))-1 (replaces the reference's sort; exact
        once the active set stabilizes).
      * w^T rebuilt into the stationary buffer via PE transpose + strided
        scatter copy.
"""

from contextlib import ExitStack

import numpy as np

import concourse.bass as bass
import concourse.tile as tile
from concourse import bacc, mybir
from concourse.bass_utils import run_bass_kernel_spmd

F32 = mybir.dt.float32
F32R = mybir.dt.float32r
I32 = mybir.dt.int32
U32 = mybir.dt.uint32
Alu = mybir.AluOpType
Act = mybir.ActivationFunctionType
AX = mybir.AxisListType

N_CORES = 8
B_FULL = 512
B_LOC = B_FULL // N_CORES   # 64
N = 256
CHUNK = 32                  # instances resident per phase
SG = 16                     # pipeline subgroup size
N_ITERS = 400
NEWTON = 3
UNROLL = 4
KAPPA = 0.1
W0 = 1.0 / N

SMALLS = ("ntau", "ptau", "s", "sq", "r", "kr", "mx", "nmx", "umx",
          "g", "cnt", "rc", "dd", "step", "nstep")
BIGS = ("w", "v", "scrA", "scrB", "scrC", "scrD")


def _emit_precompute(nc, tc, pools, c, U_d, A_d, mu_d):
    """Build G1/G2 (fp32), Frobenius norms, per-subgroup step sizes."""
    gpool, stage, psum_pre, persist = pools

    # fro partial sums per (matrix, instance): [128, CHUNK]
    fpu = persist.tile([128, CHUNK], F32, tag="fpu", name="fpu")
    fpa = persist.tile([128, CHUNK], F32, tag="fpa", name="fpa")
    ones = persist.tile([128, 1], F32, tag="ones", name="ones")
    nc.vector.memset(ones[:], 1.0)

    gt = {}
    for b in range(CHUNK):
        binst = c * CHUNK + b
        for (mat, src_d, fp) in ((1, U_d, fpu), (2, A_d, fpa)):
            # stage source as [128(k_local), 2(k_half) x 256(col)]
            ust = stage.tile([128, 2 * N], F32, tag="stage", name="stage")
            nc.sync.dma_start(
                ust[:], src_d[binst].rearrange("(two k) j -> k two j", two=2)
            )
            g = gpool.tile([128, 2 * N], F32R, tag=f"g{mat}_{b}", name=f"g{mat}_{b}")
            gt[(mat, b)] = g
            for jb in range(2):
                ps = psum_pre.tile([128, N], F32, tag="pre", name="pre")
                for kh in range(2):
                    nc.tensor.matmul(
                        ps[:],
                        lhsT=ust[:, kh * N + jb * 128 : kh * N + jb * 128 + 128],
                        rhs=ust[:, kh * N : (kh + 1) * N],
                        start=(kh == 0),
                        stop=(kh == 1),
                    )
                nc.scalar.copy(g[:, jb * N : (jb + 1) * N], ps[:])
            # squared-Frobenius partials (sum over both k-halves at once)
            sq = stage.tile([128, 2 * N], F32, tag="sqscr", name="sqscr")
            nc.scalar.activation(
                sq[:], ust[:], Act.Square, accum_out=fp[:, b : b + 1]
            )

    # partition-reduce fro partials: psum [CHUNK,1] = fp^T @ ones
    fro_u = psum_pre.tile([CHUNK, 1], F32, tag="frou", name="frou")
    fro_a = psum_pre.tile([CHUNK, 1], F32, tag="froa", name="froa")
    nc.tensor.matmul(fro_u[:], lhsT=fpu[:], rhs=ones[:], start=True, stop=True)
    nc.tensor.matmul(fro_a[:], lhsT=fpa[:], rhs=ones[:], start=True, stop=True)

    # step = 1 / (froU2 + kappa*sqrt(froA2) + 1)   -- [CHUNK,1], base 0
    sc = {nm: persist.tile([CHUNK, 1], F32, tag=nm, name=nm)
          for nm in ("stepc", "nstepc", "fa", "tmp")}
    nc.scalar.activation(sc["fa"][:], fro_a[:], Act.Sqrt)
    nc.vector.scalar_tensor_tensor(
        sc["tmp"][:], in0=sc["fa"][:], scalar=KAPPA, in1=fro_u[:],
        op0=Alu.mult, op1=Alu.add,
    )
    nc.vector.tensor_scalar_add(sc["tmp"][:], sc["tmp"][:], 1.0)
    nc.vector.reciprocal(sc["stepc"][:], sc["tmp"][:])
    nc.vector.tensor_scalar_mul(sc["nstepc"][:], sc["stepc"][:], -1.0)

    # mu chunk
    musb = persist.tile([CHUNK, N], F32, tag="musb", name="musb")
    nc.sync.dma_start(musb[:], mu_d[c * CHUNK : (c + 1) * CHUNK, :])
    return gt, sc, musb


def _emit_iteration(nc, tc, gt, stt, SW, ident, ppool):
    """One PGD iteration over the resident chunk (both subgroups)."""
    ys = {}
    # --- matvec streams (PE) ---
    for sg in range(2):
        for mat in (1, 2):
            y = ppool.tile([SG, N], F32, tag=f"y{mat}_{sg}", name=f"y{mat}_{sg}")
            ys[(mat, sg)] = y
            for b in range(SG):
                binst = sg * SG + b
                for h in range(2):
                    cb = ((h * 2 + sg) * SG + b) * SG
                    nc.tensor.matmul(
                        y[:],
                        lhsT=SW[:, cb : cb + SG],
                        rhs=gt[(mat, binst)][:, h * N : (h + 1) * N],
                        start=(b == 0 and h == 0),
                        stop=(b == SG - 1 and h == 1),
                    )

    # --- epilogues (DVE/ACT), one per subgroup ---
    for sg in range(2):
        st = stt[sg]
        y1, y2 = ys[(1, sg)], ys[(2, sg)]
        ws, vs, mus = st["w"], st["v"], st["mus"]
        ntau = st["ntau"]
        scrA, scrB, scrC, scrD = (st[k] for k in ("scrA", "scrB", "scrC", "scrD"))
        s_, sq, r, kr = (st[k] for k in ("s", "sq", "r", "kr"))
        mx, nmx, umx = (st[k] for k in ("mx", "nmx", "umx"))
        g_, cnt, rc, dd = (st[k] for k in ("g", "cnt", "rc", "dd"))
        nsteps = st["nstep"]

        # s = sum(w * y2) ; scrA = w*y2 (scratch)
        nc.vector.scalar_tensor_tensor(
            scrA[:], in0=y2[:], scalar=1.0, in1=ws[:], op0=Alu.mult,
            op1=Alu.mult, accum_out=s_[:],
        )
        # r = 1/sqrt(s + 1e-12)
        nc.scalar.activation(sq[:], s_[:], Act.Sqrt, bias=st["eps"][:])
        nc.vector.reciprocal(r[:], sq[:])
        nc.vector.tensor_scalar_mul(kr[:], r[:], KAPPA)
        # scrB = y1 - mu ; scrB += kr*y2 ; v = nstep*scrB + w
        # (each op reads at most one PSUM operand -- HW constraint)
        nc.vector.tensor_tensor(scrB[:], y1[:], mus[:], Alu.subtract)
        nc.vector.scalar_tensor_tensor(
            scrB[:], in0=y2[:], scalar=kr[:], in1=scrB[:], op0=Alu.mult,
            op1=Alu.add,
        )
        nc.vector.scalar_tensor_tensor(
            vs[:], in0=scrB[:], scalar=nsteps[:], in1=ws[:], op0=Alu.mult,
            op1=Alu.add,
        )
        # ---- projection: warm-started Newton on ntau = -tau ----
        nc.vector.tensor_reduce(mx[:], vs[:], AX.X, Alu.max)
        nc.vector.tensor_scalar_mul(nmx[:], mx[:], -1.0)
        nc.vector.tensor_scalar_add(umx[:], nmx[:], 1.0)
        nc.vector.tensor_scalar(ntau[:], ntau[:], nmx[:], umx[:], Alu.max, Alu.min)
        for _ in range(NEWTON):
            # g = sum(relu(v + ntau)) on ACT; cnt = sum(v > tau) on DVE
            # (tensor_scalar with accum_out: op1 is the REDUCTION op)
            nc.scalar.activation(scrC[:], vs[:], Act.Relu, bias=ntau[:],
                                 accum_out=g_[:])
            nc.vector.tensor_scalar_mul(st["ptau"][:], ntau[:], -1.0)
            nc.vector.tensor_scalar(
                scrD[:], vs[:], st["ptau"][:], None, Alu.is_gt, Alu.add,
                accum_out=cnt[:]
            )
            nc.vector.tensor_scalar_max(cnt[:], cnt[:], 1.0)
            nc.vector.reciprocal(rc[:], cnt[:])
            nc.vector.scalar_tensor_tensor(
                dd[:], in0=g_[:], scalar=-1.0, in1=rc[:], op0=Alu.add, op1=Alu.mult
            )
            nc.vector.tensor_tensor(ntau[:], ntau[:], dd[:], Alu.subtract)
            nc.vector.tensor_scalar(ntau[:], ntau[:], nmx[:], umx[:],
                                    Alu.max, Alu.min)
        # w_new = relu(v + ntau)
        nc.scalar.activation(ws[:], vs[:], Act.Relu, bias=ntau[:])

        # ---- rebuild stationary: w^T into SW diagonal slots ----
        wt = ppool.tile([128, 2 * SG], F32, tag=f"wt{sg}", name=f"wt{sg}")
        for h in range(2):
            nc.tensor.matmul(
                wt[:, h * SG : (h + 1) * SG],
                lhsT=ws[:, h * 128 : (h + 1) * 128],
                rhs=ident[:],
                is_transpose=True,
                skip_group_check=True,
            )
            base = (h * 2 + sg) * N
            nc.vector.tensor_scalar_add(
                SW[:, base : base + N : SG + 1], wt[:, h * SG : (h + 1) * SG], 0.0
            )


def build(n_iters=N_ITERS, unroll=UNROLL, compile=True):
    assert n_iters % unroll == 0
    nc = bacc.Bacc("TRN2", target_bir_lowering=False, debug=False)
    mu_d = nc.dram_tensor("mu", [B_LOC, N], F32, kind="ExternalInput").ap()
    U_d = nc.dram_tensor("U", [B_LOC, N, N], F32, kind="ExternalInput").ap()
    A_d = nc.dram_tensor("A", [B_LOC, N, N], F32, kind="ExternalInput").ap()
    out_d = nc.dram_tensor("out", [B_LOC, N], F32, kind="ExternalOutput").ap()

    with tile.TileContext(nc) as tc, ExitStack() as ctx:
        gpool = ctx.enter_context(tc.tile_pool(name="g", bufs=1))
        stage = ctx.enter_context(tc.tile_pool(name="stage", bufs=3))
        persist = ctx.enter_context(tc.tile_pool(name="persist", bufs=1))
        # identity [SG, SG] for PE transpose, built once via iota + is_equal
        io = persist.tile([SG, SG], I32, tag="io", name="io")
        ident = persist.tile([SG, SG], F32, tag="ident", name="ident")
        nc.gpsimd.iota(io[:], [[1, SG]], channel_multiplier=-1)
        nc.vector.tensor_scalar(ident[:], io[:], 0, None, Alu.is_equal)

        for c in range(B_LOC // CHUNK):
            with tc.tile_pool(name=f"pp{c}", bufs=2, space="PSUM") as psum_pre:
                gt, sc, musb = _emit_precompute(
                    nc, tc, (gpool, stage, psum_pre, persist), c, U_d, A_d, mu_d
                )

            # persistent per-subgroup iteration state (all base-partition 0)
            SW = persist.tile([128, 4 * N], F32R, tag="SW", name="SW")
            # Memset rejects the f32r dtype at codegen: write exact bit
            # patterns through a uint32 view (0.0 and 1/256 are exactly
            # representable, so f32r rounding is a no-op)
            nc.vector.memset(SW[:].bitcast(U32), 0)
            wini = persist.tile([128, SG], F32R, tag="wini", name="wini")
            nc.vector.memset(wini[:].bitcast(U32), np.float32(W0).view(np.uint32).item())
            for hsg in range(4):
                nc.vector.tensor_scalar_add(
                    SW[:, hsg * N : hsg * N + N : SG + 1], wini[:], 0.0
                )

            stt = []
            for sg in range(2):
                st = {}
                for nm in BIGS:
                    st[nm] = persist.tile([SG, N], F32, tag=f"{nm}{sg}",
                                          name=f"{nm}{sg}")
                for nm in SMALLS + ("eps",):
                    st[nm] = persist.tile([SG, 1], F32, tag=f"{nm}{sg}",
                                          name=f"{nm}{sg}")
                st["mus"] = persist.tile([SG, N], F32, tag=f"mus{sg}",
                                         name=f"mus{sg}")
                nc.vector.memset(st["w"][:], W0)
                nc.vector.memset(st["ntau"][:], 0.0)
                nc.vector.memset(st["eps"][:], 1e-12)
                # per-subgroup copies of step/nstep/mu (cross-partition: DMA)
                if sg == 0:
                    nc.vector.tensor_scalar_add(st["step"][:], sc["stepc"][0:SG, :], 0.0)
                    nc.vector.tensor_scalar_add(st["nstep"][:], sc["nstepc"][0:SG, :], 0.0)
                    nc.vector.tensor_scalar_add(st["mus"][:], musb[0:SG, :], 0.0)
                else:
                    nc.sync.dma_start(st["step"][:], sc["stepc"][SG : 2 * SG, :])
                    nc.sync.dma_start(st["nstep"][:], sc["nstepc"][SG : 2 * SG, :])
                    nc.sync.dma_start(st["mus"][:], musb[SG : 2 * SG, :])
                stt.append(st)

            with tc.tile_pool(name=f"pmv{c}", bufs=1, space="PSUM") as ppool:
                with tc.For_i(0, n_iters // unroll, 1,
                              hint_engines=(mybir.EngineType.PE,)):
                    for _ in range(unroll):
                        _emit_iteration(nc, tc, gt, stt, SW, ident, ppool)

            # final: renormalize w and write out
            for sg in range(2):
                st = stt[sg]
                sumw = persist.tile([SG, 1], F32, tag=f"sumw{sg}", name=f"sumw{sg}")
                rs = persist.tile([SG, 1], F32, tag=f"rs{sg}", name=f"rs{sg}")
                outb = persist.tile([SG, N], F32, tag=f"outb{sg}", name=f"outb{sg}")
                nc.vector.tensor_reduce(sumw[:], st["w"][:], AX.X, Alu.add)
                nc.vector.tensor_scalar_add(sumw[:], sumw[:], 1e-12)
                nc.vector.reciprocal(rs[:], sumw[:])
                nc.vector.tensor_scalar(outb[:], st["w"][:], rs[:], None, Alu.mult)
                nc.sync.dma_start(
                    out_d[c * CHUNK + sg * SG : c * CHUNK + (sg + 1) * SG, :],
                    outb[:],
                )

    if compile:
        nc.compile()
    return nc


_nc_cache = {}


def _get_nc(n_iters=N_ITERS):
    if n_iters not in _nc_cache:
        _nc_cache[n_iters] = build(n_iters)
    return _nc_cache[n_iters]


def kernel(mu: np.ndarray, U: np.ndarray, A: np.ndarray) -> np.ndarray:
    nc = _get_nc()
    in_maps = [
        {
            "mu": np.ascontiguousarray(mu[i * B_LOC : (i + 1) * B_LOC]),
            "U": np.ascontiguousarray(U[i * B_LOC : (i + 1) * B_LOC]),
            "A": np.ascontiguousarray(A[i * B_LOC : (i + 1) * B_LOC]),
        }
        for i in range(N_CORES)
    ]
    res = run_bass_kernel_spmd(nc, in_maps, core_ids=list(range(N_CORES)))
    return np.concatenate([res.results[i]["out"] for i in range(N_CORES)], axis=0)
